# revision 31
# baseline (speedup 1.0000x reference)
"""Differential multi-head attention (DiffAttn) Trainium2 kernel.

Self-contained: accepts FULL inputs, shards across 8 NeuronCores
(data-parallel over batch x tensor-parallel over head pairs), runs a
Bass/Tile kernel per core, gathers partial outputs on host.
"""

import numpy as np
from contextlib import ExitStack

import concourse.bass as bass
import concourse.tile as tile
from concourse import bacc, mybir
from concourse import bass_utils
from concourse.masks import make_identity

B, S, D, H = 2, 2048, 2048, 16
HD = D // H            # 128
HALF = H // 2          # 8
DH2 = HD // 2          # 64
THETA = 500000.0
EPS_RMS = 1e-6
EPS_HN = 1e-5
NCORES = 8
NPAIR = 2              # head pairs per core
NSLOT = 4              # head slots per core (2 pairs x 2 branches)
CW = NSLOT * HD        # 512 projection columns per core
DT = D // 128          # 16 contraction tiles
ST = S // 128          # 16 sequence tiles
QB = 4                 # q blocks of 512
QBW = S // QB          # 512

f32 = mybir.dt.float32
f32r = mybir.dt.float32r
AF = mybir.ActivationFunctionType
ALU = mybir.AluOpType

_prog_cache = {}


def _bcast_head(ap_2d, nh):
    """[128, 128] AP -> [128, nh(step0), 128] broadcast over head dim."""
    return bass.AP(
        tensor=ap_2d.tensor,
        offset=ap_2d.offset,
        ap=[ap_2d.ap[0], [0, nh], ap_2d.ap[1]],
    )


def _build_program():
    nc = bacc.Bacc("TRN2", target_bir_lowering=False, debug=False)

    x_d = nc.dram_tensor("x", [S, D], f32, kind="ExternalInput").ap()
    wq_d = nc.dram_tensor("wq", [D, CW], f32r, kind="ExternalInput").ap()
    wk_d = nc.dram_tensor("wk", [D, CW], f32r, kind="ExternalInput").ap()
    wv_d = nc.dram_tensor("wv", [D, CW], f32r, kind="ExternalInput").ap()
    wo_d = nc.dram_tensor("wo", [NPAIR * HD, D], f32r, kind="ExternalInput").ap()
    cos_d = nc.dram_tensor("cosf", [S, HD], f32, kind="ExternalInput").ap()
    sin_d = nc.dram_tensor("sinf", [S, HD], f32, kind="ExternalInput").ap()
    out_d = nc.dram_tensor("out", [S, D], f32, kind="ExternalOutput").ap()

    with tile.TileContext(nc) as tc, ExitStack() as ctx:
        # ---- pools ----
        consts = ctx.enter_context(tc.tile_pool(name="consts", bufs=1))
        dram = ctx.enter_context(tc.tile_pool(name="dram", bufs=1, space="DRAM"))

        ctxA = ctx.enter_context(ExitStack())
        psB = ctxA.enter_context(tc.tile_pool(name="psB", bufs=2, space="PSUM"))
        wpool = ctxA.enter_context(tc.tile_pool(name="wpool", bufs=1))
        xpool = ctxA.enter_context(tc.tile_pool(name="xpool", bufs=3))
        sqpool = ctxA.enter_context(tc.tile_pool(name="sqpool", bufs=1))
        spool = ctxA.enter_context(tc.tile_pool(name="spool", bufs=4))
        xspool = ctxA.enter_context(tc.tile_pool(name="xspool", bufs=1))
        xntpool = ctxA.enter_context(tc.tile_pool(name="xntpool", bufs=1))
        ropep = ctxA.enter_context(tc.tile_pool(name="ropep", bufs=2))
        drainp = ctxA.enter_context(tc.tile_pool(name="drainp", bufs=2))
        psA = ctxA.enter_context(tc.tile_pool(name="psA", bufs=2, space="PSUM"))
        psQ = ctxA.enter_context(tc.tile_pool(name="psQ", bufs=2, space="PSUM"))

        # ---- constants ----
        ident = consts.tile([128, 128], f32)
        make_identity(nc, ident)
        ident_r = consts.tile([128, 128], f32r)
        nc.scalar.copy(out=ident_r, in_=ident)
        ones_f = consts.tile([128, 1], f32)
        nc.vector.memset(ones_f, 1.0)
        ones_r = consts.tile([128, 1], f32r)
        nc.scalar.copy(out=ones_r, in_=ones_f)
        eps2 = consts.tile([128, 1], f32)
        nc.vector.memset(eps2, 2.0 * EPS_RMS)
        epshn = consts.tile([128, 1], f32)
        nc.vector.memset(epshn, EPS_HN)

        cosf = consts.tile([128, ST, HD], f32)
        nc.sync.dma_start(out=cosf, in_=cos_d.rearrange("(t p) c -> p t c", p=128))
        sinf = consts.tile([128, ST, HD], f32)
        nc.sync.dma_start(out=sinf, in_=sin_d.rearrange("(t p) c -> p t c", p=128))

        wq = wpool.tile([128, DT, CW], f32r, name="wq_sb")
        nc.sync.dma_start(out=wq, in_=wq_d.rearrange("(t p) c -> p t c", p=128))
        wk = wpool.tile([128, DT, CW], f32r, name="wk_sb")
        nc.sync.dma_start(out=wk, in_=wk_d.rearrange("(t p) c -> p t c", p=128))
        wv = wpool.tile([128, DT, CW], f32r, name="wv_sb")
        nc.sync.dma_start(out=wv, in_=wv_d.rearrange("(t p) c -> p t c", p=128))

        # ---- DRAM spill for qT / kT / v ----
        l_d = dram.tile([NPAIR, QB, 2, QBW], f32)
        qT_d = dram.tile([NSLOT, 128, S], f32r)
        kT_d = dram.tile([NSLOT, 128, S], f32r)
        v_d = dram.tile([ST, 128, CW], f32r)

        # ================= PHASE A =================
        # PE warm-up: keep TensorE busy while initial DMAs stream so the
        # HAM clock-gate opens before real matmuls arrive.
        junkf = consts.tile([128, 512], f32)
        nc.vector.memset(junkf, 0.5)
        junkr = consts.tile([128, 512], f32r)
        nc.scalar.copy(out=junkr, in_=junkf)
        pw = psA.tile([128, 512], f32, tag="ptx", name="pw")
        NWARM = 56
        for i in range(NWARM):
            nc.tensor.matmul(pw, ident_r, junkr, start=(i == 0), stop=(i == NWARM - 1))
        junk_sb = consts.tile([1, 1], f32)
        nc.scalar.copy(out=junk_sb, in_=pw[0:1, 0:1])
        nc.gpsimd.dma_start(out=l_d[0, 0, 0, 0:1], in_=junk_sb)

        HC = 8          # half-chunks of 2 stiles
        for hc in range(HC):
            xnT = xntpool.tile([128, DT, 256], f32r, tag="xnT")
            for sl in range(2):
                g = hc * 2 + sl
                X = xpool.tile([128, D], f32, tag="X")
                nc.sync.dma_start(out=X, in_=x_d[g * 128:(g + 1) * 128, :])
                # rmsnorm stats: ssq = sum((x + 2e-6)^2)
                xsq = sqpool.tile([128, D], f32, tag="xsq")
                ssq = spool.tile([128, 1], f32, tag="ssq")
                nc.scalar.activation(out=xsq, in_=X, func=AF.Square,
                                     bias=eps2, scale=1.0, accum_out=ssq)
                nr = spool.tile([128, 1], f32, tag="nr")
                nc.scalar.activation(out=nr, in_=ssq, func=AF.Sqrt, scale=float(D))
                nr2 = spool.tile([128, 1], f32, tag="nr2")
                nc.vector.tensor_scalar(out=nr2, in0=nr, scalar1=EPS_RMS,
                                        scalar2=None, op0=ALU.add)
                rinv = spool.tile([128, 1], f32, tag="rinv")
                nc.vector.reciprocal(out=rinv, in_=nr2)
                # xa_scaled = (x + eps) * rinv   (f32r)
                XS = xspool.tile([128, D], f32r, tag="XS")
                nc.vector.tensor_scalar(out=XS, in0=X, scalar1=EPS_RMS,
                                        scalar2=rinv, op0=ALU.add, op1=ALU.mult)
                # transpose 16 [128,128] blocks -> xnT[:, dt, sl*128:+128]
                for dt4 in range(4):
                    ptx = psA.tile([128, 512], f32r, tag="ptx")
                    for k in range(4):
                        dt = dt4 * 4 + k
                        nc.tensor.transpose(ptx[:, k * 128:(k + 1) * 128],
                                            XS[:, dt * 128:(dt + 1) * 128], ident_r)
                    dst = xnT[:, dt4 * 4:(dt4 + 1) * 4, sl * 128:(sl + 1) * 128]
                    nc.scalar.copy(out=dst, in_=ptx.rearrange("p (k c) -> p k c", k=4))

            # projections for the 2 stiles of this half-chunk
            for sl in range(2):
                g = hc * 2 + sl
                for proj, W in (("q", wq), ("k", wk), ("v", wv)):
                    pp = psB.tile([128, CW], f32, tag="pp")
                    for dt in range(DT):
                        nc.tensor.matmul(pp, xnT[:, dt, sl * 128:(sl + 1) * 128],
                                         W[:, dt, :], start=(dt == 0), stop=(dt == DT - 1))
                    if proj == "v":
                        vsb = drainp.tile([128, CW], f32r, tag="vsb")
                        nc.scalar.copy(out=vsb, in_=pp)
                        nc.sync.dma_start(out=v_d[g], in_=vsb)
                    else:
                        # rope in natural layout [s=128, 4 heads x 128]
                        pv = pp.rearrange("p (h c) -> p h c", h=NSLOT)
                        cos_b = _bcast_head(cosf[:, g, :], NSLOT)
                        sin_b = _bcast_head(sinf[:, g, :], NSLOT)
                        tmp = ropep.tile([128, NSLOT, HD], f32, tag="ropetmp")
                        rotc = ropep.tile([128, NSLOT, HD], f32, tag="ropec")
                        rot = ropep.tile([128, NSLOT, HD], f32r, tag="roper")
                        nc.vector.tensor_mul(out=tmp[:, :, 0:DH2],
                                             in0=pv[:, :, DH2:HD], in1=sin_b[:, :, 0:DH2])
                        nc.vector.tensor_mul(out=tmp[:, :, DH2:HD],
                                             in0=pv[:, :, 0:DH2], in1=sin_b[:, :, DH2:HD])
                        nc.vector.tensor_mul(out=rotc, in0=pv, in1=cos_b)
                        nc.vector.tensor_add(out=rot, in0=rotc, in1=tmp)
                        ptq = psQ.tile([128, CW], f32r, tag="ptq")
                        for h4 in range(NSLOT):
                            nc.tensor.transpose(ptq[:, h4 * 128:(h4 + 1) * 128],
                                                rot[:, h4, :], ident_r)
                        qsb = drainp.tile([128, NSLOT, HD], f32r, tag="qsb")
                        nc.scalar.copy(out=qsb,
                                       in_=ptq.rearrange("p (h c) -> p h c", h=NSLOT))
                        dstT = qT_d if proj == "q" else kT_d
                        for h4 in range(NSLOT):
                            nc.sync.dma_start(out=dstT[h4][:, g * 128:(g + 1) * 128],
                                              in_=qsb[:, h4, :])

        # ================= PHASE B =================
        ctxA.close()
        wopool = ctx.enter_context(tc.tile_pool(name="wopool", bufs=1))
        znpool = ctx.enter_context(tc.tile_pool(name="znpool", bufs=1))
        SCALE = 1.0 / float(np.sqrt(HD).astype(np.float32))

        ctxB = ctx.enter_context(ExitStack())
        bpool = ctxB.enter_context(tc.tile_pool(name="bpool", bufs=2))
        qpool = ctxB.enter_context(tc.tile_pool(name="qpool", bufs=3))
        ppool = ctxB.enter_context(tc.tile_pool(name="ppool", bufs=6))
        opool = ctxB.enter_context(tc.tile_pool(name="opool", bufs=3))
        epool = ctxB.enter_context(tc.tile_pool(name="epool", bufs=3))
        psS = ctxB.enter_context(tc.tile_pool(name="psS", bufs=2, space="PSUM"))
        psO = ctxB.enter_context(tc.tile_pool(name="psO", bufs=1, space="PSUM"))
        psL = ctxB.enter_context(tc.tile_pool(name="psL", bufs=1, space="PSUM"))
        assert QBW == 512

        wo_sb = wopool.tile([128, NPAIR, D], f32r, name="wo_sb")
        nc.sync.dma_start(out=wo_sb, in_=wo_d.rearrange("(j c) e -> c j e", c=128))

        znT = znpool.tile([128, NPAIR, S], f32r, name="znT")

        SCALE = 1.0 / float(np.sqrt(HD).astype(np.float32))

        for j in range(NPAIR):
            s1, s2 = j, j + 2
            kT1 = bpool.tile([128, S], f32r, tag="kT1")
            kT2 = bpool.tile([128, S], f32r, tag="kT2")
            v1 = bpool.tile([128, ST, HD], f32r, tag="v1")
            v2 = bpool.tile([128, ST, HD], f32r, tag="v2")

            for qb in range(QB):
                cs = slice(qb * QBW, (qb + 1) * QBW)
                ts4 = slice(qb * 4, qb * 4 + 4)
                nc.sync.dma_start(out=kT1[:, cs], in_=kT_d[s1][:, cs])
                nc.gpsimd.dma_start(out=kT2[:, cs], in_=kT_d[s2][:, cs])
                nc.sync.dma_start(
                    out=v1[:, ts4, :],
                    in_=v_d[ts4, :, s1 * HD:(s1 + 1) * HD].rearrange("t p c -> p t c"))
                nc.gpsimd.dma_start(
                    out=v2[:, ts4, :],
                    in_=v_d[ts4, :, s2 * HD:(s2 + 1) * HD].rearrange("t p c -> p t c"))

            for qb in range(QB):
                qT1 = qpool.tile([128, QBW], f32r, tag="qT1")
                nc.sync.dma_start(out=qT1, in_=qT_d[s1][:, qb * QBW:(qb + 1) * QBW])
                qT2 = qpool.tile([128, QBW], f32r, tag="qT2")
                nc.sync.dma_start(out=qT2, in_=qT_d[s2][:, qb * QBW:(qb + 1) * QBW])

                po = [psO.tile([128, QBW], f32, tag=f"po{br}", name=f"po{br}")
                      for br in range(2)]
                pl = [psL.tile([1, QBW], f32, tag=f"pl{br}", name=f"pl{br}")
                      for br in range(2)]
                T = 4 * qb + 4
                for t in range(T):
                    crossing = t >= 4 * qb
                    c0 = 128 * (t - 4 * qb) if crossing else 0
                    for br, (kT, qT, vv) in enumerate(((kT1, qT1, v1), (kT2, qT2, v2))):
                        if j == 0 and qb == 0:
                            kslice = kp[br][:, t * 128:(t + 1) * 128]
                            vslice = vp[br][:, t, :]
                        else:
                            kslice = kT[:, t * 128:(t + 1) * 128]
                            vslice = vv[:, t, :]
                        ss = psS.tile([128, QBW], f32, tag="ss")
                        nc.tensor.matmul(ss[:, c0:], kslice,
                                         qT[:, c0:], start=True, stop=True)
                        pT = ppool.tile([128, QBW], f32r, tag="pT")
                        nc.scalar.activation(out=pT[:, c0:], in_=ss[:, c0:],
                                             func=AF.Exp, scale=SCALE)
                        if crossing:
                            nc.gpsimd.affine_select(
                                out=pT[:, c0:], in_=pT[:, c0:], compare_op=ALU.is_ge,
                                fill=0.0, base=0, pattern=[[1, QBW - c0]],
                                channel_multiplier=-1)
                        nc.tensor.matmul(pl[br][:, c0:], ones_r, pT[:, c0:],
                                         start=(t == 0), stop=(t == T - 1),
                                         skip_group_check=True)
                        nc.tensor.matmul(po[br][:, c0:], vslice, pT[:, c0:],
                                         start=(t == 0), stop=(t == T - 1),
                                         skip_group_check=True)

                # ---- epilogue ----
                for br in range(2):
                    lsb = epool.tile([1, QBW], f32, tag=f"lsb{br}")
                    nc.scalar.copy(out=lsb, in_=pl[br])
                    nc.gpsimd.dma_start(out=l_d[j, qb, br], in_=lsb)
                lT = epool.tile([128, 2, 4], f32, tag="lT")
                nc.sync.dma_start(
                    out=lT,
                    in_=l_d[j, qb].rearrange("br (qs q) -> q br qs", q=128))
                linv = epool.tile([128, 2, 4], f32, tag="linv")
                nc.vector.reciprocal(out=linv, in_=lT)
                linvT = [linv[:, 0, :], linv[:, 1, :]]

                oc = []
                for br in range(2):
                    o = opool.tile([128, QBW], f32, tag=f"oc{br}")
                    nc.vector.tensor_copy(out=o, in_=po[br])
                    oc.append(o)

                pt = []
                for br in range(2):
                    p = psS.tile([128, QBW], f32, tag="ss", name=f"pt{br}")
                    for qs in range(4):
                        nc.tensor.transpose(p[:, qs * 128:(qs + 1) * 128],
                                            oc[br][:, qs * 128:(qs + 1) * 128], ident)
                    pt.append(p)

                ptz = psS.tile([128, QBW], f32r, tag="ss")
                for qs in range(4):
                    sl_ = slice(qs * 128, (qs + 1) * 128)
                    a = epool.tile([128, 128], f32, tag="ea")
                    nc.vector.tensor_scalar(out=a, in0=pt[0][:, sl_],
                                            scalar1=linvT[0][:, qs:qs + 1],
                                            scalar2=None, op0=ALU.mult)
                    b2 = epool.tile([128, 128], f32, tag="eb")
                    nc.vector.tensor_scalar(out=b2, in0=pt[1][:, sl_],
                                            scalar1=linvT[1][:, qs:qs + 1],
                                            scalar2=None, op0=ALU.mult)
                    z = epool.tile([128, 128], f32, tag="ez")
                    nc.vector.tensor_sub(out=z, in0=a, in1=b2)
                    st = epool.tile([128, nc.vector.BN_STATS_DIM], f32, tag="est")
                    nc.vector.bn_stats(out=st, in_=z)
                    mv = epool.tile([128, nc.vector.BN_AGGR_DIM], f32, tag="emv")
                    nc.vector.bn_aggr(out=mv, in_=st)
                    sd = epool.tile([128, 1], f32, tag="esd")
                    nc.scalar.activation(out=sd, in_=mv[:, 1:2], func=AF.Sqrt,
                                         bias=epshn)
                    rstd = epool.tile([128, 1], f32, tag="erstd")
                    nc.vector.reciprocal(out=rstd, in_=sd)
                    zn = epool.tile([128, 128], f32r, tag="ezn")
                    nc.vector.tensor_scalar(out=zn, in0=z, scalar1=mv[:, 0:1],
                                            scalar2=rstd, op0=ALU.subtract, op1=ALU.mult)
                    nc.tensor.transpose(ptz[:, sl_], zn, ident_r)
                nc.scalar.copy(out=znT[:, j, qb * QBW:(qb + 1) * QBW], in_=ptz)

        # ---- output projection ----
        ctxB.close()
        psP = ctx.enter_context(tc.tile_pool(name="psP", bufs=2, space="PSUM"))
        opool2 = ctx.enter_context(tc.tile_pool(name="opool2", bufs=3))
        for st16 in range(ST):
            for ec in range(4):
                pso = psP.tile([128, 512], f32, tag="pso")
                for j in range(NPAIR):
                    nc.tensor.matmul(pso, znT[:, j, st16 * 128:(st16 + 1) * 128],
                                     wo_sb[:, j, ec * 512:(ec + 1) * 512],
                                     start=(j == 0), stop=(j == NPAIR - 1))
                osb = opool2.tile([128, 512], f32, tag="osb")
                if (st16 + ec) % 2 == 0:
                    nc.scalar.copy(out=osb, in_=pso)
                else:
                    nc.vector.tensor_copy(out=osb, in_=pso)
                ring = nc.sync if (st16 + ec) % 2 == 0 else nc.gpsimd
                ring.dma_start(
                    out=out_d[st16 * 128:(st16 + 1) * 128, ec * 512:(ec + 1) * 512],
                    in_=osb)

    nc.compile()
    return nc


def _host_prep(inputs):
    """Fold params, build rope tables, shard into 8 per-core input maps."""
    f = {k: np.asarray(v) for k, v in inputs.items()}
    x = f["x"].astype(np.float32)
    g = f["g"].astype(np.float32)
    gamma = f["gamma"].reshape(H).astype(np.float32)
    beta = f["beta"].reshape(H).astype(np.float32)
    wo = f["wo"].astype(np.float32)
    bo = f["bo"].astype(np.float32)

    # lambda (host, fp32 like reference)
    qk1 = np.sum(f["lambda_q1"].astype(np.float32) * f["lambda_k1"].astype(np.float32),
                 axis=-1)
    qk2 = np.sum(f["lambda_q2"].astype(np.float32) * f["lambda_k2"].astype(np.float32),
                 axis=-1)
    lam = np.clip(np.exp(qk1) - np.exp(qk2) + f["lambda_init"].astype(np.float32),
                  0.0, 1.0)[0, :HALF]          # [8]

    wq_g = f["wq"].astype(np.float32) * g[:, None]
    wk_g = f["wk"].astype(np.float32) * g[:, None]
    wv_g = f["wv"].astype(np.float32) * g[:, None]

    assert not f["bq"].any() and not f["bk"].any() and not f["bv"].any(), \
        "nonzero q/k/v biases not supported"

    # rope tables (fp32, mirroring the reference math)
    positions = f["positions"].astype(np.float32)
    inv_freq = (1.0 / (np.float32(THETA) **
                       (np.arange(0, HD, 2, dtype=np.float32) / np.float32(HD))))
    ph = positions[:, None] * inv_freq[None, :]          # [S, 64]
    cos = np.cos(ph).astype(np.float32)
    sin = np.sin(ph).astype(np.float32)
    cosf = np.concatenate([cos, cos], axis=1)            # [S, 128]
    sinf = np.concatenate([-sin, sin], axis=1)           # [S, 128]

    in_maps = []
    for core in range(NCORES):
        b, pg = core // 4, core % 4
        p0, p1 = 2 * pg, 2 * pg + 1
        slots = [p0, p1, p0 + 8, p1 + 8]

        def cols(w):
            return np.concatenate([w[:, h * HD:(h + 1) * HD] for h in slots], axis=1)

        wv_c = cols(wv_g).copy()
        wv_c[:, 2 * HD:3 * HD] *= lam[p0]
        wv_c[:, 3 * HD:4 * HD] *= lam[p1]

        wo_eff = np.concatenate(
            [gamma[p] * wo[p * HD:(p + 1) * HD, :]
             + gamma[p + 8] * wo[(p + 8) * HD:(p + 9) * HD, :]
             for p in (p0, p1)], axis=0)                  # [256, D]

        in_maps.append({
            "x": np.ascontiguousarray(x[b]),
            "wq": np.ascontiguousarray(cols(wq_g)),
            "wk": np.ascontiguousarray(cols(wk_g)),
            "wv": np.ascontiguousarray(wv_c),
            "wo": np.ascontiguousarray(wo_eff),
            "cosf": cosf,
            "sinf": sinf,
        })

    bo_eff = bo.astype(np.float64).copy()
    for h in range(H):
        bo_eff += float(beta[h]) * wo[h * HD:(h + 1) * HD, :].astype(np.float64).sum(0)
    return in_maps, bo_eff


def _maybe_enable_trace():
    """Optional NTFF profiling (dev only, KERNEL_TRACE=1). Best-effort."""
    import os
    if not os.environ.get("KERNEL_TRACE"):
        return False
    try:
        import types, sys
        import antenv
        if "antenv.axon_hooks" not in sys.modules:
            from trn_agent_boot.trn_boot import _ntff_profile_via_ctypes
            hookmod = types.ModuleType("antenv.axon_hooks")
            hook = _ntff_profile_via_ctypes("/opt/axon/libaxon_pjrt.so")
            hookmod.get_axon_ntff_profile_hook = lambda: hook
            hookmod.set_axon_ntff_profile_hook = lambda h: None
            sys.modules["antenv.axon_hooks"] = hookmod
            antenv.axon_hooks = hookmod
        bass_utils.upload_artifacts = lambda d: d
        return True
    except Exception:
        return False


def kernel(**inputs):
    in_maps, bo_eff = _host_prep(inputs)
    if "nc" not in _prog_cache:
        _prog_cache["nc"] = _build_program()
    nc = _prog_cache["nc"]
    trace = _maybe_enable_trace()
    res = bass_utils.run_bass_kernel_spmd(nc, in_maps, core_ids=list(range(NCORES)),
                                          trace=trace)
    _prog_cache["exec_time_ns"] = res.exec_time_ns
    _prog_cache["results"] = res
    out = np.zeros((B, S, D), dtype=np.float64)
    for core in range(NCORES):
        out[core // 4] += res.results[core]["out"].astype(np.float64)
    out += bo_eff[None, None, :]
    return out.astype(np.float32)


# revision 33
# speedup vs baseline: 1.0679x; 1.0679x over previous
"""Differential multi-head attention (DiffAttn) Trainium2 kernel.

Self-contained: accepts FULL inputs, shards across 8 NeuronCores
(data-parallel over batch x tensor-parallel over head pairs), runs a
Bass/Tile kernel per core, gathers partial outputs on host.
"""

import numpy as np
from contextlib import ExitStack

import concourse.bass as bass
import concourse.tile as tile
from concourse import bacc, mybir
from concourse import bass_utils
from concourse.masks import make_identity

B, S, D, H = 2, 2048, 2048, 16
HD = D // H            # 128
HALF = H // 2          # 8
DH2 = HD // 2          # 64
THETA = 500000.0
EPS_RMS = 1e-6
EPS_HN = 1e-5
NCORES = 8
NPAIR = 2              # head pairs per core
NSLOT = 4              # head slots per core (2 pairs x 2 branches)
CW = NSLOT * HD        # 512 projection columns per core
DT = D // 128          # 16 contraction tiles
ST = S // 128          # 16 sequence tiles
QB = 4                 # q blocks of 512
QBW = S // QB          # 512

f32 = mybir.dt.float32
f32r = mybir.dt.float32r
AF = mybir.ActivationFunctionType
ALU = mybir.AluOpType

_prog_cache = {}


def _bcast_head(ap_2d, nh):
    """[128, 128] AP -> [128, nh(step0), 128] broadcast over head dim."""
    return bass.AP(
        tensor=ap_2d.tensor,
        offset=ap_2d.offset,
        ap=[ap_2d.ap[0], [0, nh], ap_2d.ap[1]],
    )


def _build_program():
    nc = bacc.Bacc("TRN2", target_bir_lowering=False, debug=False)

    x_d = nc.dram_tensor("x", [S, D], f32, kind="ExternalInput").ap()
    wq_d = nc.dram_tensor("wq", [D, CW], f32r, kind="ExternalInput").ap()
    wk_d = nc.dram_tensor("wk", [D, CW], f32r, kind="ExternalInput").ap()
    wv_d = nc.dram_tensor("wv", [D, CW], f32r, kind="ExternalInput").ap()
    wo_d = nc.dram_tensor("wo", [NPAIR * HD, D], f32r, kind="ExternalInput").ap()
    cos_d = nc.dram_tensor("cosf", [S, HD], f32, kind="ExternalInput").ap()
    sin_d = nc.dram_tensor("sinf", [S, HD], f32, kind="ExternalInput").ap()
    out_d = nc.dram_tensor("out", [S, D], f32, kind="ExternalOutput").ap()

    with tile.TileContext(nc) as tc, ExitStack() as ctx:
        # ---- pools ----
        consts = ctx.enter_context(tc.tile_pool(name="consts", bufs=1))
        dram = ctx.enter_context(tc.tile_pool(name="dram", bufs=1, space="DRAM"))

        ctxA = ctx.enter_context(ExitStack())
        psB = ctxA.enter_context(tc.tile_pool(name="psB", bufs=2, space="PSUM"))
        wpool = ctxA.enter_context(tc.tile_pool(name="wpool", bufs=1))
        xpool = ctxA.enter_context(tc.tile_pool(name="xpool", bufs=2))
        sqpool = ctxA.enter_context(tc.tile_pool(name="sqpool", bufs=1))
        spool = ctxA.enter_context(tc.tile_pool(name="spool", bufs=4))
        xspool = ctxA.enter_context(tc.tile_pool(name="xspool", bufs=1))
        xntpool = ctxA.enter_context(tc.tile_pool(name="xntpool", bufs=1))
        ropep = ctxA.enter_context(tc.tile_pool(name="ropep", bufs=2))
        drainp = ctxA.enter_context(tc.tile_pool(name="drainp", bufs=2))
        psA = ctxA.enter_context(tc.tile_pool(name="psA", bufs=2, space="PSUM"))
        psQ = ctxA.enter_context(tc.tile_pool(name="psQ", bufs=2, space="PSUM"))

        # ---- constants ----
        ident = consts.tile([128, 128], f32)
        make_identity(nc, ident)
        ident_r = consts.tile([128, 128], f32r)
        nc.scalar.copy(out=ident_r, in_=ident)
        ones_f = consts.tile([128, 1], f32)
        nc.vector.memset(ones_f, 1.0)
        ones_r = consts.tile([128, 1], f32r)
        nc.scalar.copy(out=ones_r, in_=ones_f)
        eps2 = consts.tile([128, 1], f32)
        nc.vector.memset(eps2, 2.0 * EPS_RMS)
        epshn = consts.tile([128, 1], f32)
        nc.vector.memset(epshn, EPS_HN)

        cosf = consts.tile([128, ST, HD], f32)
        nc.sync.dma_start(out=cosf, in_=cos_d.rearrange("(t p) c -> p t c", p=128))
        sinf = consts.tile([128, ST, HD], f32)
        nc.sync.dma_start(out=sinf, in_=sin_d.rearrange("(t p) c -> p t c", p=128))

        wq = wpool.tile([128, DT, CW], f32r, name="wq_sb")
        nc.sync.dma_start(out=wq, in_=wq_d.rearrange("(t p) c -> p t c", p=128))
        wk = wpool.tile([128, DT, CW], f32r, name="wk_sb")
        nc.sync.dma_start(out=wk, in_=wk_d.rearrange("(t p) c -> p t c", p=128))
        wv = wpool.tile([128, DT, CW], f32r, name="wv_sb")
        nc.sync.dma_start(out=wv, in_=wv_d.rearrange("(t p) c -> p t c", p=128))

        # ---- DRAM spill for qT / kT / v ----
        l_d = dram.tile([NPAIR, QB, 2, QBW], f32)
        scr_d = dram.tile([1, 4], f32)
        qT_d = dram.tile([NSLOT, 128, S], f32r)
        kT_d = dram.tile([NSLOT, 128, S], f32r)
        v_d = dram.tile([ST, 128, CW], f32r)

        # ================= PHASE A =================
        # PE warm-up: keep TensorE busy while initial DMAs stream so the
        # HAM clock-gate opens before real matmuls arrive.
        junkf = consts.tile([128, 512], f32)
        nc.vector.memset(junkf, 0.5)
        junkr = consts.tile([128, 512], f32r)
        nc.scalar.copy(out=junkr, in_=junkf)
        pw = psA.tile([128, 512], f32, tag="ptx", name="pw")
        NWARM = 56
        for i in range(NWARM):
            nc.tensor.matmul(pw, ident_r, junkr, start=(i == 0), stop=(i == NWARM - 1))
        junk_sb = consts.tile([1, 1], f32)
        nc.scalar.copy(out=junk_sb, in_=pw[0:1, 0:1])
        nc.gpsimd.dma_start(out=l_d[0, 0, 0, 0:1], in_=junk_sb)

        HC = 8          # half-chunks of 2 stiles
        for hc in range(HC):
            xnT = xntpool.tile([128, DT, 256], f32r, tag="xnT")
            for sl in range(2):
                g = hc * 2 + sl
                X = xpool.tile([128, D], f32, tag="X")
                nc.sync.dma_start(out=X, in_=x_d[g * 128:(g + 1) * 128, :])
                # rmsnorm stats: ssq = sum((x + 2e-6)^2)
                xsq = sqpool.tile([128, D], f32, tag="xsq")
                ssq = spool.tile([128, 1], f32, tag="ssq")
                nc.scalar.activation(out=xsq, in_=X, func=AF.Square,
                                     bias=eps2, scale=1.0, accum_out=ssq)
                nr = spool.tile([128, 1], f32, tag="nr")
                nc.scalar.activation(out=nr, in_=ssq, func=AF.Sqrt, scale=float(D))
                nr2 = spool.tile([128, 1], f32, tag="nr2")
                nc.vector.tensor_scalar(out=nr2, in0=nr, scalar1=EPS_RMS,
                                        scalar2=None, op0=ALU.add)
                rinv = spool.tile([128, 1], f32, tag="rinv")
                nc.vector.reciprocal(out=rinv, in_=nr2)
                # xa_scaled = (x + eps) * rinv   (f32r)
                XS = xspool.tile([128, D], f32r, tag="XS")
                nc.vector.tensor_scalar(out=XS, in0=X, scalar1=EPS_RMS,
                                        scalar2=rinv, op0=ALU.add, op1=ALU.mult)
                # transpose 16 [128,128] blocks -> xnT[:, dt, sl*128:+128]
                for dt4 in range(4):
                    ptx = psA.tile([128, 512], f32r, tag="ptx")
                    for k in range(4):
                        dt = dt4 * 4 + k
                        nc.tensor.transpose(ptx[:, k * 128:(k + 1) * 128],
                                            XS[:, dt * 128:(dt + 1) * 128], ident_r)
                    dst = xnT[:, dt4 * 4:(dt4 + 1) * 4, sl * 128:(sl + 1) * 128]
                    nc.scalar.copy(out=dst, in_=ptx.rearrange("p (k c) -> p k c", k=4))

            # projections for the 2 stiles of this half-chunk
            for sl in range(2):
                g = hc * 2 + sl
                for proj, W in (("q", wq), ("k", wk), ("v", wv)):
                    pp = psB.tile([128, CW], f32, tag="pp")
                    for dt in range(DT):
                        nc.tensor.matmul(pp, xnT[:, dt, sl * 128:(sl + 1) * 128],
                                         W[:, dt, :], start=(dt == 0), stop=(dt == DT - 1))
                    if proj == "v":
                        vsb = drainp.tile([128, CW], f32r, tag="vsb")
                        nc.scalar.copy(out=vsb, in_=pp)
                        nc.sync.dma_start(out=v_d[g], in_=vsb)
                    else:
                        # rope in natural layout [s=128, 4 heads x 128]
                        pv = pp.rearrange("p (h c) -> p h c", h=NSLOT)
                        cos_b = _bcast_head(cosf[:, g, :], NSLOT)
                        sin_b = _bcast_head(sinf[:, g, :], NSLOT)
                        tmp = ropep.tile([128, NSLOT, HD], f32, tag="ropetmp")
                        rotc = ropep.tile([128, NSLOT, HD], f32, tag="ropec")
                        rot = ropep.tile([128, NSLOT, HD], f32r, tag="roper")
                        nc.vector.tensor_mul(out=tmp[:, :, 0:DH2],
                                             in0=pv[:, :, DH2:HD], in1=sin_b[:, :, 0:DH2])
                        nc.vector.tensor_mul(out=tmp[:, :, DH2:HD],
                                             in0=pv[:, :, 0:DH2], in1=sin_b[:, :, DH2:HD])
                        nc.vector.tensor_mul(out=rotc, in0=pv, in1=cos_b)
                        nc.vector.tensor_add(out=rot, in0=rotc, in1=tmp)
                        ptq = psQ.tile([128, CW], f32r, tag="ptq")
                        for h4 in range(NSLOT):
                            nc.tensor.transpose(ptq[:, h4 * 128:(h4 + 1) * 128],
                                                rot[:, h4, :], ident_r)
                        qsb = drainp.tile([128, NSLOT, HD], f32r, tag="qsb")
                        nc.scalar.copy(out=qsb,
                                       in_=ptq.rearrange("p (h c) -> p h c", h=NSLOT))
                        dstT = qT_d if proj == "q" else kT_d
                        for h4 in range(NSLOT):
                            nc.sync.dma_start(out=dstT[h4][:, g * 128:(g + 1) * 128],
                                              in_=qsb[:, h4, :])

        # ================= PHASE B =================
        ctxA.close()
        wopool = ctx.enter_context(tc.tile_pool(name="wopool", bufs=1))
        znpool = ctx.enter_context(tc.tile_pool(name="znpool", bufs=1))
        SCALE = 1.0 / float(np.sqrt(HD).astype(np.float32))

        ctxB = ctx.enter_context(ExitStack())
        bpool = ctxB.enter_context(tc.tile_pool(name="bpool", bufs=2))
        qpool = ctxB.enter_context(tc.tile_pool(name="qpool", bufs=3))
        ppool = ctxB.enter_context(tc.tile_pool(name="ppool", bufs=6))
        opool = ctxB.enter_context(tc.tile_pool(name="opool", bufs=3))
        epool = ctxB.enter_context(tc.tile_pool(name="epool", bufs=3))
        psS = ctxB.enter_context(tc.tile_pool(name="psS", bufs=2, space="PSUM"))
        psO = ctxB.enter_context(tc.tile_pool(name="psO", bufs=1, space="PSUM"))
        psL = ctxB.enter_context(tc.tile_pool(name="psL", bufs=1, space="PSUM"))
        assert QBW == 512

        wo_sb = wopool.tile([128, NPAIR, D], f32r, name="wo_sb")
        nc.sync.dma_start(out=wo_sb, in_=wo_d.rearrange("(j c) e -> c j e", c=128))

        znT = znpool.tile([128, NPAIR, S], f32r, name="znT")

        SCALE = 1.0 / float(np.sqrt(HD).astype(np.float32))

        for j in range(NPAIR):
            s1, s2 = j, j + 2
            kT1 = bpool.tile([128, S], f32r, tag="kT1")
            kT2 = bpool.tile([128, S], f32r, tag="kT2")
            v1 = bpool.tile([128, ST, HD], f32r, tag="v1")
            v2 = bpool.tile([128, ST, HD], f32r, tag="v2")

            for qb in range(QB):
                cs = slice(qb * QBW, (qb + 1) * QBW)
                ts4 = slice(qb * 4, qb * 4 + 4)
                nc.sync.dma_start(out=kT1[:, cs], in_=kT_d[s1][:, cs])
                nc.gpsimd.dma_start(out=kT2[:, cs], in_=kT_d[s2][:, cs])
                nc.sync.dma_start(
                    out=v1[:, ts4, :],
                    in_=v_d[ts4, :, s1 * HD:(s1 + 1) * HD].rearrange("t p c -> p t c"))
                nc.gpsimd.dma_start(
                    out=v2[:, ts4, :],
                    in_=v_d[ts4, :, s2 * HD:(s2 + 1) * HD].rearrange("t p c -> p t c"))

            for qb in range(QB):
                qT1 = qpool.tile([128, QBW], f32r, tag="qT1")
                nc.sync.dma_start(out=qT1, in_=qT_d[s1][:, qb * QBW:(qb + 1) * QBW])
                qT2 = qpool.tile([128, QBW], f32r, tag="qT2")
                nc.sync.dma_start(out=qT2, in_=qT_d[s2][:, qb * QBW:(qb + 1) * QBW])

                po = [psO.tile([128, QBW], f32, tag=f"po{br}", name=f"po{br}")
                      for br in range(2)]
                pl = [psL.tile([1, QBW], f32, tag=f"pl{br}", name=f"pl{br}")
                      for br in range(2)]
                T = 4 * qb + 4
                for t in range(T):
                    crossing = t >= 4 * qb
                    c0 = 128 * (t - 4 * qb) if crossing else 0
                    for br, (kT, qT, vv) in enumerate(((kT1, qT1, v1), (kT2, qT2, v2))):
                        if j == 0 and qb == 0:
                            kslice = kp[br][:, t * 128:(t + 1) * 128]
                            vslice = vp[br][:, t, :]
                        else:
                            kslice = kT[:, t * 128:(t + 1) * 128]
                            vslice = vv[:, t, :]
                        ss = psS.tile([128, QBW], f32, tag="ss")
                        nc.tensor.matmul(ss[:, c0:], kslice,
                                         qT[:, c0:], start=True, stop=True)
                        pT = ppool.tile([128, QBW], f32r, tag="pT")
                        nc.scalar.activation(out=pT[:, c0:], in_=ss[:, c0:],
                                             func=AF.Exp, scale=SCALE)
                        if crossing:
                            nc.gpsimd.affine_select(
                                out=pT[:, c0:], in_=pT[:, c0:], compare_op=ALU.is_ge,
                                fill=0.0, base=0, pattern=[[1, QBW - c0]],
                                channel_multiplier=-1)
                        nc.tensor.matmul(pl[br][:, c0:], ones_r, pT[:, c0:],
                                         start=(t == 0), stop=(t == T - 1),
                                         skip_group_check=True)
                        nc.tensor.matmul(po[br][:, c0:], vslice, pT[:, c0:],
                                         start=(t == 0), stop=(t == T - 1),
                                         skip_group_check=True)

                # ---- epilogue ----
                for br in range(2):
                    lsb = epool.tile([1, QBW], f32, tag=f"lsb{br}")
                    nc.scalar.copy(out=lsb, in_=pl[br])
                    nc.gpsimd.dma_start(out=l_d[j, qb, br], in_=lsb)
                lT = epool.tile([128, 2, 4], f32, tag="lT")
                nc.sync.dma_start(
                    out=lT,
                    in_=l_d[j, qb].rearrange("br (qs q) -> q br qs", q=128))
                linv = epool.tile([128, 2, 4], f32, tag="linv")
                nc.vector.reciprocal(out=linv, in_=lT)
                linvT = [linv[:, 0, :], linv[:, 1, :]]

                oc = []
                for br in range(2):
                    o = opool.tile([128, QBW], f32, tag=f"oc{br}")
                    nc.vector.tensor_copy(out=o, in_=po[br])
                    oc.append(o)

                pt = []
                for br in range(2):
                    p = psS.tile([128, QBW], f32, tag="ss", name=f"pt{br}")
                    for qs in range(4):
                        nc.tensor.transpose(p[:, qs * 128:(qs + 1) * 128],
                                            oc[br][:, qs * 128:(qs + 1) * 128], ident)
                    pt.append(p)

                ptz = psS.tile([128, QBW], f32r, tag="ss")
                for qs in range(4):
                    sl_ = slice(qs * 128, (qs + 1) * 128)
                    a = epool.tile([128, 128], f32, tag="ea")
                    nc.vector.tensor_scalar(out=a, in0=pt[0][:, sl_],
                                            scalar1=linvT[0][:, qs:qs + 1],
                                            scalar2=None, op0=ALU.mult)
                    b2 = epool.tile([128, 128], f32, tag="eb")
                    nc.vector.tensor_scalar(out=b2, in0=pt[1][:, sl_],
                                            scalar1=linvT[1][:, qs:qs + 1],
                                            scalar2=None, op0=ALU.mult)
                    z = epool.tile([128, 128], f32, tag="ez")
                    nc.vector.tensor_sub(out=z, in0=a, in1=b2)
                    st = epool.tile([128, nc.vector.BN_STATS_DIM], f32, tag="est")
                    nc.vector.bn_stats(out=st, in_=z)
                    mv = epool.tile([128, nc.vector.BN_AGGR_DIM], f32, tag="emv")
                    nc.vector.bn_aggr(out=mv, in_=st)
                    sd = epool.tile([128, 1], f32, tag="esd")
                    nc.scalar.activation(out=sd, in_=mv[:, 1:2], func=AF.Sqrt,
                                         bias=epshn)
                    rstd = epool.tile([128, 1], f32, tag="erstd")
                    nc.vector.reciprocal(out=rstd, in_=sd)
                    zn = epool.tile([128, 128], f32r, tag="ezn")
                    nc.vector.tensor_scalar(out=zn, in0=z, scalar1=mv[:, 0:1],
                                            scalar2=rstd, op0=ALU.subtract, op1=ALU.mult)
                    nc.tensor.transpose(ptz[:, sl_], zn, ident_r)
                nc.scalar.copy(out=znT[:, j, qb * QBW:(qb + 1) * QBW], in_=ptz)

        # ---- output projection ----
        ctxB.close()
        psP = ctx.enter_context(tc.tile_pool(name="psP", bufs=2, space="PSUM"))
        opool2 = ctx.enter_context(tc.tile_pool(name="opool2", bufs=3))
        for st16 in range(ST):
            for ec in range(4):
                pso = psP.tile([128, 512], f32, tag="pso")
                for j in range(NPAIR):
                    nc.tensor.matmul(pso, znT[:, j, st16 * 128:(st16 + 1) * 128],
                                     wo_sb[:, j, ec * 512:(ec + 1) * 512],
                                     start=(j == 0), stop=(j == NPAIR - 1))
                osb = opool2.tile([128, 512], f32, tag="osb")
                if (st16 + ec) % 2 == 0:
                    nc.scalar.copy(out=osb, in_=pso)
                else:
                    nc.vector.tensor_copy(out=osb, in_=pso)
                ring = nc.sync if (st16 + ec) % 2 == 0 else nc.gpsimd
                ring.dma_start(
                    out=out_d[st16 * 128:(st16 + 1) * 128, ec * 512:(ec + 1) * 512],
                    in_=osb)

    nc.compile()
    return nc


def _host_prep(inputs):
    """Fold params, build rope tables, shard into 8 per-core input maps."""
    f = {k: np.asarray(v) for k, v in inputs.items()}
    x = f["x"].astype(np.float32)
    g = f["g"].astype(np.float32)
    gamma = f["gamma"].reshape(H).astype(np.float32)
    beta = f["beta"].reshape(H).astype(np.float32)
    wo = f["wo"].astype(np.float32)
    bo = f["bo"].astype(np.float32)

    # lambda (host, fp32 like reference)
    qk1 = np.sum(f["lambda_q1"].astype(np.float32) * f["lambda_k1"].astype(np.float32),
                 axis=-1)
    qk2 = np.sum(f["lambda_q2"].astype(np.float32) * f["lambda_k2"].astype(np.float32),
                 axis=-1)
    lam = np.clip(np.exp(qk1) - np.exp(qk2) + f["lambda_init"].astype(np.float32),
                  0.0, 1.0)[0, :HALF]          # [8]

    wq_g = f["wq"].astype(np.float32) * g[:, None]
    wk_g = f["wk"].astype(np.float32) * g[:, None]
    wv_g = f["wv"].astype(np.float32) * g[:, None]

    assert not f["bq"].any() and not f["bk"].any() and not f["bv"].any(), \
        "nonzero q/k/v biases not supported"

    # rope tables (fp32, mirroring the reference math)
    positions = f["positions"].astype(np.float32)
    inv_freq = (1.0 / (np.float32(THETA) **
                       (np.arange(0, HD, 2, dtype=np.float32) / np.float32(HD))))
    ph = positions[:, None] * inv_freq[None, :]          # [S, 64]
    cos = np.cos(ph).astype(np.float32)
    sin = np.sin(ph).astype(np.float32)
    cosf = np.concatenate([cos, cos], axis=1)            # [S, 128]
    sinf = np.concatenate([-sin, sin], axis=1)           # [S, 128]

    in_maps = []
    for core in range(NCORES):
        b, pg = core // 4, core % 4
        p0, p1 = 2 * pg, 2 * pg + 1
        slots = [p0, p1, p0 + 8, p1 + 8]

        def cols(w):
            return np.concatenate([w[:, h * HD:(h + 1) * HD] for h in slots], axis=1)

        wv_c = cols(wv_g).copy()
        wv_c[:, 2 * HD:3 * HD] *= lam[p0]
        wv_c[:, 3 * HD:4 * HD] *= lam[p1]

        wo_eff = np.concatenate(
            [gamma[p] * wo[p * HD:(p + 1) * HD, :]
             + gamma[p + 8] * wo[(p + 8) * HD:(p + 9) * HD, :]
             for p in (p0, p1)], axis=0)                  # [256, D]

        in_maps.append({
            "x": np.ascontiguousarray(x[b]),
            "wq": np.ascontiguousarray(cols(wq_g)),
            "wk": np.ascontiguousarray(cols(wk_g)),
            "wv": np.ascontiguousarray(wv_c),
            "wo": np.ascontiguousarray(wo_eff),
            "cosf": cosf,
            "sinf": sinf,
        })

    bo_eff = bo.astype(np.float64).copy()
    for h in range(H):
        bo_eff += float(beta[h]) * wo[h * HD:(h + 1) * HD, :].astype(np.float64).sum(0)
    return in_maps, bo_eff


def _maybe_enable_trace():
    """Optional NTFF profiling (dev only, KERNEL_TRACE=1). Best-effort."""
    import os
    if not os.environ.get("KERNEL_TRACE"):
        return False
    try:
        import types, sys
        import antenv
        if "antenv.axon_hooks" not in sys.modules:
            from trn_agent_boot.trn_boot import _ntff_profile_via_ctypes
            hookmod = types.ModuleType("antenv.axon_hooks")
            hook = _ntff_profile_via_ctypes("/opt/axon/libaxon_pjrt.so")
            hookmod.get_axon_ntff_profile_hook = lambda: hook
            hookmod.set_axon_ntff_profile_hook = lambda h: None
            sys.modules["antenv.axon_hooks"] = hookmod
            antenv.axon_hooks = hookmod
        bass_utils.upload_artifacts = lambda d: d
        return True
    except Exception:
        return False


def kernel(**inputs):
    in_maps, bo_eff = _host_prep(inputs)
    if "nc" not in _prog_cache:
        _prog_cache["nc"] = _build_program()
    nc = _prog_cache["nc"]
    trace = _maybe_enable_trace()
    res = bass_utils.run_bass_kernel_spmd(nc, in_maps, core_ids=list(range(NCORES)),
                                          trace=trace)
    _prog_cache["exec_time_ns"] = res.exec_time_ns
    _prog_cache["results"] = res
    out = np.zeros((B, S, D), dtype=np.float64)
    for core in range(NCORES):
        out[core // 4] += res.results[core]["out"].astype(np.float64)
    out += bo_eff[None, None, :]
    return out.astype(np.float32)
